# revision 61
# baseline (speedup 1.0000x reference)
"""Trainium2 Bass kernel for nn_Attention_18786186952997.

Dense causal-attention transformer block with ternarized (BitNet-style)
weights and RoPE:

    wq = ternarize(w_qkv); wp = ternarize(w_proj)
    qkv = x @ wq.T ; q,k,v split ; RoPE(q,k) ; causal SDPA ; y @ wp.T

Sharding: 8 cores = 2 batches x 4 head-groups (4 heads each).  Each core
computes its batch's qkv projections for its 4 heads, runs causal
flash-style attention fully on-chip, and produces a partial (transposed)
projection output; the host sums the 4 partials per batch.

Device compute layout is channel-major ("transposed"): q.T/k.T are
produced as [head_dim, tokens].  Head dims are host-permuted so the
rotate_half partition swap becomes an adjacent-lane DVE stream_shuffle.
exp(scores.T) is exactly the stationary layout that A@V needs; softmax
denominators come free from ones-columns appended to V.  Ternary weights
are passed as exact bf16 sign matrices; the abs-mean scales are folded
into the exp() scale and a host-side output scale.

Pipeline structure: the ScalarE exp chain is the pacer; scores matmuls
run one chunk ahead, qkv/v/proj matmuls fill TensorE stalls, and PSUM is
partitioned so attention accumulation never blocks the filler matmuls.
"""

import os
import sys
import types

import numpy as np

sys.path.insert(0, "/opt/trn_rl_repo")

import ml_dtypes  # noqa: E402

BF16 = ml_dtypes.bfloat16

B, T, C, H, D = 2, 2048, 1024, 16, 64
N_CORES = 8
HEADS_PER_CORE = 4
P = 128
QT = 512            # q tile (moving free dim)
NQT = T // QT       # 4
NKC = T // P        # 16 k chunks
NCC = C // P        # 8 contraction chunks
KC_ORDER = [2, 3, 0, 1, 4, 5, 6, 7]  # xt DMA arrival order (6,7 land last)
ADD_ENGINE = "vector"
SWAP_MASK = [i ^ 1 for i in range(32)]

_CACHE = {}


def _install_ntff_hook():
    """bass_utils' trace=True path needs antenv.axon_hooks, absent in this
    image; synthesize it around the boot module's ctypes hook."""
    if "antenv.axon_hooks" in sys.modules:
        return
    try:
        import antenv  # noqa: F401
        from trn_agent_boot.trn_boot import _ntff_profile_via_ctypes
    except Exception:
        return
    mod = types.ModuleType("antenv.axon_hooks")
    holder = {}
    mod.set_axon_ntff_profile_hook = lambda h: holder.__setitem__("h", h)
    mod.get_axon_ntff_profile_hook = lambda: holder.get("h")
    sys.modules["antenv.axon_hooks"] = mod
    sys.modules["antenv"].axon_hooks = mod
    try:
        hook = _ntff_profile_via_ctypes("/opt/axon/libaxon_pjrt.so")
        mod.set_axon_ntff_profile_hook(hook)
    except Exception:
        pass


def _ternarize_host(w):
    """Sign matrix and abs-mean scale, bit-matching the jax reference."""
    try:
        import jax.numpy as jnp

        wj = jnp.asarray(w)
        am = jnp.maximum(jnp.abs(wj).mean(), 1e-5)
        thr = 0.7 * am
        s = jnp.where(wj > thr, 1.0, jnp.where(wj < -thr, -1.0, 0.0))
        return np.asarray(s, dtype=np.float32), np.float32(am)
    except Exception:
        am = np.float32(max(np.abs(w).astype(np.float32).mean(dtype=np.float32), 1e-5))
        thr = np.float32(0.7) * am
        s = np.where(w > thr, 1.0, np.where(w < -thr, -1.0, 0.0)).astype(np.float32)
        return s, am


def _build_program():
    import concourse.bass as bass  # noqa: F401
    import concourse.mybir as mybir
    import concourse.tile as tile
    from concourse import bacc

    F32 = mybir.dt.float32
    BF = mybir.dt.bfloat16
    AF = mybir.ActivationFunctionType

    nc = bacc.Bacc("TRN2", target_bir_lowering=False, debug=False,
                   num_devices=N_CORES)

    # all inputs partition-major so DMA descriptors are 128 x contiguous-KB
    xt = nc.dram_tensor("xt", [P, NCC, T], BF, kind="ExternalInput").ap()
    wqk = nc.dram_tensor("wqk", [P, NCC, 512], BF, kind="ExternalInput").ap()
    wv = nc.dram_tensor("wv", [P, NCC, 256], BF, kind="ExternalInput").ap()
    wp = nc.dram_tensor("wp", [P, 2, 1024], BF, kind="ExternalInput").ap()
    cosp = nc.dram_tensor("cosp", [D, T], BF, kind="ExternalInput").ap()
    ssp = nc.dram_tensor("ssp", [D, T], BF, kind="ExternalInput").ap()
    sc_exp = nc.dram_tensor("sc_exp", [P, 1], F32, kind="ExternalInput").ap()
    outT = nc.dram_tensor("outT", [P, NQT, 8, QT], BF,
                          kind="ExternalOutput").ap()

    with tile.TileContext(nc) as tc:
        with (
            tc.tile_pool(name="consts", bufs=1) as consts,
            tc.tile_pool(name="tmps", bufs=3) as tmps,
            tc.tile_pool(name="epool", bufs=6) as epool,
            tc.tile_pool(name="opool", bufs=3) as opool,
            tc.tile_pool(name="ps_sc", bufs=2, space="PSUM") as ps_sc,
            tc.tile_pool(name="ps_y", bufs=1, space="PSUM") as ps_y,
            tc.tile_pool(name="ps_mm", bufs=2, space="PSUM") as ps_mm,
        ):
            # ---- persistent SBUF loads ----
            # sync + scalar are fast HWDGE rings (~175GB/s); gpsimd is a
            # slow software ring (~50GB/s) - only late-needed tensors there
            xt_sb = consts.tile([P, NCC, T], BF)
            wqk_sb = consts.tile([P, NCC, 512], BF)
            wv_sb = consts.tile([P, NCC, 256], BF)
            wp_sb = consts.tile([P, 2, 1024], BF)
            sce_sb = consts.tile([P, 1], F32)
            cos_sb = consts.tile([P, T], BF)
            ss_sb = consts.tile([P, T], BF)
            nc.sync.dma_start(out=wqk_sb, in_=wqk[:])
            nc.scalar.dma_start(out=sce_sb, in_=sc_exp[:])
            nc.scalar.dma_start(out=cos_sb[0:64, :], in_=cosp[:])
            nc.scalar.dma_start(out=ss_sb[0:64, :], in_=ssp[:])
            nc.gpsimd.dma_start(out=xt_sb[:, 6:8, :], in_=xt[:, 6:8, :])
            nc.scalar.dma_start(out=xt_sb[:, 0:2, :], in_=xt[:, 0:2, :])
            nc.sync.dma_start(out=xt_sb[:, 2:4, :], in_=xt[:, 2:4, :])
            nc.scalar.dma_start(out=xt_sb[:, 4:6, :], in_=xt[:, 4:6, :])
            nc.sync.dma_start(out=wv_sb, in_=wv[:])
            nc.gpsimd.dma_start(out=wp_sb, in_=wp[:])
            # no-dep DVE work first (the dup copies below wait on their DMAs
            # at queue head; gpsimd is occupied ~20us by software-DGE)
            warm_sb = consts.tile([P, QT], BF)
            nc.vector.memset(warm_sb, 0.0)
            qk_sb = consts.tile([P, 4, T], BF)  # blk: q01, q23, k01, k23
            v_sb = consts.tile([P, NKC, 2, 256], BF)
            y_sb = consts.tile([P, 2, T], BF)
            # per head: [ones(64) | v(64)] -> denominators at psum rows 0:64
            v_sb4 = v_sb.rearrange("p n g (h o d) -> p n g h o d", h=2, o=2)
            nc.vector.memset(v_sb4[:, :, :, :, 0, :], 1.0)
            # duplicate cos/ss into rows 64:128 on-chip (halves ring traffic)
            nc.vector.tensor_copy(cos_sb[64:128, :], cos_sb[0:64, :])
            nc.vector.tensor_copy(ss_sb[64:128, :], ss_sb[0:64, :])

            # warm the exp table while the inputs stream in
            dummy = consts.tile([1, 1], F32)
            nc.scalar.activation(dummy, sce_sb[0:1, 0:1], AF.Exp)

            def emit_rope(ps, dest, qs):
                # scores at the next qt boundary wait on this chain; lift it
                # above older DVE work (epilogues, casts) in the scheduler
                with tc.high_priority(offset=150):
                    t1 = tmps.tile([P, QT], F32, tag="t1")
                    nc.vector.tensor_mul(t1, ps, cos_sb[:, qs])
                    tsh = tmps.tile([P, QT], F32, tag="tsh")
                    nc.vector.stream_shuffle(tsh, ps, SWAP_MASK)
                    t2 = tmps.tile([P, QT], F32, tag="t2")
                    nc.vector.tensor_mul(t2, tsh, ss_sb[:, qs])
                    if ADD_ENGINE == "vector":
                        nc.vector.tensor_add(dest, t1, t2)
                    else:
                        nc.gpsimd.tensor_add(dest, t1, t2)

            def emit_qkv_pair(qt, pair):
                # wqk col blocks: q01 [0:128) q23 [128:256) k01 [256:384) k23 [384:512)
                # j0/j1 chains interleave so consecutive matmuls alternate
                # PSUM banks (fill/drain overlap)
                qs = slice(qt * QT, (qt + 1) * QT)
                ps0 = ps_mm.tile([P, QT], F32, tag="mm", name="qkvps0")
                ps1 = ps_mm.tile([P, QT], F32, tag="mm", name="qkvps1")
                for i, kc in enumerate(KC_ORDER):
                    for j, ps in ((0, ps0), (1, ps1)):
                        base = 128 * (pair + 2 * j)
                        nc.tensor.matmul(
                            ps,
                            lhsT=wqk_sb[:, kc, base:base + P],
                            rhs=xt_sb[:, kc, qs],
                            start=(i == 0),
                            stop=(i == NCC - 1),
                        )
                emit_rope(ps0, qk_sb[:, pair, qs], qs)
                emit_rope(ps1, qk_sb[:, 2 + pair, qs], qs)

            def emit_v2(tt0):
                # two token-tiles interleaved: alternating PSUM banks
                vpA = ps_mm.tile([P, 256], F32, tag="mm", name="vpsA")
                vpB = ps_mm.tile([P, 256], F32, tag="mm", name="vpsB")
                for i, kc in enumerate(KC_ORDER):
                    for tt, vp in ((tt0, vpA), (tt0 + 1, vpB)):
                        nc.tensor.matmul(
                            vp,
                            lhsT=xt_sb[:, kc, tt * P:(tt + 1) * P],
                            rhs=wv_sb[:, kc, :],
                            start=(i == 0),
                            stop=(i == NCC - 1),
                        )
                for tt, vp in ((tt0, vpA), (tt0 + 1, vpB)):
                    vp4 = vp.rearrange("p (g h d) -> p g h d", g=2, h=2)
                    nc.vector.tensor_copy(v_sb4[:, tt, :, :, 1, :], vp4)

            def emit_attn(grp, qt):
                q_t = qk_sb[:, grp, :]
                k_t = qk_sb[:, 2 + grp, :]
                qs = slice(qt * QT, (qt + 1) * QT)
                KC = 4 * (qt + 1)  # causal k chunks
                yacc = ps_y.tile([P, 1024], F32, tag="y", name="yacc")
                for kc in range(KC):
                    ks = slice(kc * P, (kc + 1) * P)
                    delta = kc * P - qt * QT
                    d0 = max(delta, 0)
                    ps = ps_sc.tile([P, 1024], F32, tag="sc", name="scps")
                    p2 = ps.rearrange("p (j f) -> p j f", j=2)
                    nc.tensor.matmul(p2[:, 0, d0:QT], lhsT=k_t[0:64, ks],
                                     rhs=q_t[0:64, qt * QT + d0:(qt + 1) * QT],
                                     start=True, stop=True)
                    nc.tensor.matmul(p2[:, 1, d0:QT], lhsT=k_t[64:128, ks],
                                     rhs=q_t[64:128, qt * QT + d0:(qt + 1) * QT],
                                     start=True, stop=True)
                    e = epool.tile([P, 1024], BF, tag="e")
                    e2 = e.rearrange("p (j f) -> p j f", j=2)
                    if d0 == 0:
                        nc.scalar.activation(e, ps, AF.Exp,
                                             scale=sce_sb[:, 0:1])
                    else:
                        nc.scalar.activation(e2[:, :, d0:QT],
                                             p2[:, :, d0:QT],
                                             AF.Exp, scale=sce_sb[:, 0:1])
                    if delta >= 0:
                        # triangular mask only on the 128-wide diagonal band
                        nc.gpsimd.affine_select(
                            e2[:, :, delta:delta + P],
                            e2[:, :, delta:delta + P],
                            pattern=[[0, 2], [1, P]],
                            compare_op=mybir.AluOpType.is_ge,
                            fill=0.0,
                            base=0,
                            channel_multiplier=-1,
                        )
                    nc.tensor.matmul(yacc[:, d0:QT],
                                     lhsT=v_sb[:, kc, grp, 0:128],
                                     rhs=e[:, d0:QT],
                                     start=(kc == 0), stop=(kc == KC - 1))
                    nc.tensor.matmul(yacc[:, QT + d0:1024],
                                     lhsT=v_sb[:, kc, grp, 128:256],
                                     rhs=e[:, QT + d0:1024],
                                     start=(kc == 0), stop=(kc == KC - 1))
                # both heads: denom rows 0:64, y rows 64:128
                rcA = tmps.tile([P, QT], F32, tag="rc")
                nc.vector.reciprocal_approx_fast(rcA[0:64, :], yacc[0:64, 0:QT])
                nc.vector.tensor_mul(y_sb[0:64, grp, qs], yacc[64:128, 0:QT],
                                     rcA[0:64, :])
                rcB = tmps.tile([P, QT], F32, tag="rc")
                nc.vector.reciprocal_approx_fast(rcB[0:64, :],
                                                 yacc[0:64, QT:1024])
                nc.vector.tensor_mul(y_sb[64:128, grp, qs],
                                     yacc[64:128, QT:1024],
                                     rcB[0:64, :])

            def emit_proj(qt):
                qs = slice(qt * QT, (qt + 1) * QT)
                ot = opool.tile([P, 8, QT], BF, tag="ot")
                for mt0 in range(0, 8, 2):
                    ppA = ps_mm.tile([P, QT], F32, tag="mm", name="projpsA")
                    ppB = ps_mm.tile([P, QT], F32, tag="mm", name="projpsB")
                    for ch in range(2):
                        for mt, pp in ((mt0, ppA), (mt0 + 1, ppB)):
                            ms = slice(mt * P, (mt + 1) * P)
                            nc.tensor.matmul(pp, lhsT=wp_sb[:, ch, ms],
                                             rhs=y_sb[:, ch, qs],
                                             start=(ch == 0), stop=(ch == 1))
                    if qt == 3:
                        # post-exp-chain: split casts across ScalarE + DVE
                        nc.vector.tensor_copy(ot[:, mt0, :], ppA)
                        nc.scalar.activation(ot[:, mt0 + 1, :], ppB, AF.Copy)
                    else:
                        nc.vector.tensor_copy(ot[:, mt0, :], ppA)
                        nc.vector.tensor_copy(ot[:, mt0 + 1, :], ppB)
                    if mt0 == 2:
                        nc.sync.dma_start(out=outT[:, qt, 0:4, :],
                                          in_=ot[:, 0:4, :])
                nc.sync.dma_start(out=outT[:, qt, 4:8, :], in_=ot[:, 4:8, :])

            # HAM warm-up: keep the PE busy while inputs stream in, so the
            # first real matmuls run at full clock (results never read)
            wps = ps_sc.tile([P, 1024], F32, tag="sc", name="warm")
            for _ in range(48):
                nc.tensor.matmul(wps[:, 0:QT], lhsT=warm_sb[:, 0:P],
                                 rhs=warm_sb, start=True, stop=True)

            # ---- pipeline: exp chain paces; qkv/v/proj fill PE stalls ----
            emit_qkv_pair(0, 0)
            emit_v2(0)
            emit_qkv_pair(0, 1)
            emit_v2(2)
            emit_attn(0, 0)
            emit_attn(1, 0)
            emit_qkv_pair(1, 0)
            emit_qkv_pair(1, 1)
            emit_v2(4)
            emit_v2(6)
            emit_proj(0)
            emit_attn(0, 1)
            emit_qkv_pair(2, 0)
            emit_attn(1, 1)
            emit_qkv_pair(2, 1)
            emit_v2(8)
            emit_v2(10)
            emit_attn(0, 2)
            emit_qkv_pair(3, 0)
            emit_attn(1, 2)
            emit_qkv_pair(3, 1)
            emit_v2(12)
            emit_v2(14)
            emit_attn(0, 3)
            # late filler: the qt=3 stretch is scalar-paced with no qkv/v
            # work left, so park proj(1)/proj(2) matmuls here
            emit_proj(1)
            emit_proj(2)
            emit_attn(1, 3)
            emit_proj(3)

    nc.finalize()
    return nc


def _prep_inputs(x, cos, sin, w_qkv, w_proj):
    sq, am_q = _ternarize_host(w_qkv)
    sp, am_p = _ternarize_host(w_proj)

    # head-dim interleave: partition p holds logical dim order64[p] so
    # rotate_half becomes an adjacent-lane swap (stream_shuffle)
    order64 = np.empty(D, dtype=np.int64)
    order64[0::2] = np.arange(D // 2)
    order64[1::2] = np.arange(D // 2) + D // 2

    cos_t = np.ascontiguousarray(cos[0, 0].T).astype(np.float32)  # [D, T]
    sin_t = np.ascontiguousarray(sin[0, 0].T).astype(np.float32)
    sgn = np.where(np.arange(D)[:, None] < D // 2, np.float32(-1.0),
                   np.float32(1.0))
    ss_t = sin_t * sgn
    cosp = np.ascontiguousarray(cos_t[order64]).astype(BF16)      # [D, T]
    ssp = np.ascontiguousarray(ss_t[order64]).astype(BF16)
    sc_exp = np.full((P, 1), am_q * am_q / np.sqrt(np.float32(D)),
                     np.float32)
    sc_out = np.float32(am_q) * np.float32(am_p)

    def pmajor(a):
        # [R*128, M] -> [128, R, M]: partition-major so each partition's
        # DMA source is one contiguous run
        r = a.shape[0] // P
        return np.ascontiguousarray(
            a.reshape(r, P, a.shape[1]).transpose(1, 0, 2))

    in_maps = []
    for core in range(N_CORES):
        b, g = divmod(core, HEADS_PER_CORE)
        heads = [4 * g + h for h in range(4)]
        qk_rows = np.concatenate(
            [h * D + order64 for h in heads])          # permuted q/k rows
        v_rows = np.concatenate(
            [np.arange(h * D, (h + 1) * D) for h in heads])  # natural
        wqk_block = np.concatenate([sq[qk_rows], sq[C + qk_rows]], axis=0)
        wqk_t = pmajor(np.ascontiguousarray(wqk_block.T).astype(BF16))
        wv_t = pmajor(np.ascontiguousarray(sq[2 * C + v_rows].T).astype(BF16))
        wp_t = pmajor(np.ascontiguousarray(sp[:, v_rows].T).astype(BF16))
        xt = pmajor(np.ascontiguousarray(x[b].T).astype(BF16))
        in_maps.append({
            "xt": xt, "wqk": wqk_t, "wv": wv_t, "wp": wp_t,
            "cosp": cosp, "ssp": ssp, "sc_exp": sc_exp,
        })
    return in_maps, sc_out


def kernel(x, cos, sin, w_qkv, w_proj):
    x = np.asarray(x, dtype=np.float32)
    cos = np.asarray(cos, dtype=np.float32)
    sin = np.asarray(sin, dtype=np.float32)
    w_qkv = np.asarray(w_qkv, dtype=np.float32)
    w_proj = np.asarray(w_proj, dtype=np.float32)

    _install_ntff_hook()
    from concourse.bass_utils import run_bass_kernel_spmd

    if "nc" not in _CACHE:
        _CACHE["nc"] = _build_program()
    nc = _CACHE["nc"]

    in_maps, sc_out = _prep_inputs(x, cos, sin, w_qkv, w_proj)
    trace = bool(os.environ.get("KERNEL_TRACE"))
    res = run_bass_kernel_spmd(nc, in_maps, core_ids=list(range(N_CORES)),
                               trace=trace)
    _CACHE["exec_time_ns"] = res.exec_time_ns

    out = np.zeros((B, T, C), dtype=np.float32)
    for core in range(N_CORES):
        b = core // HEADS_PER_CORE
        # outT: [P, NQT, 8, QT] -> full [C, T] is [mt*P + p, qt*QT + t]
        o = res.results[core]["outT"].astype(np.float32)
        o = o.transpose(2, 0, 1, 3).reshape(C, T)
        out[b] += o.T
    out *= sc_out
    return out
